# revision 5
# baseline (speedup 1.0000x reference)
"""Trainium2 Bass kernel for nn_Attn_6545530159401.

Computation (reference):
    enc  = encoder_outputs.transpose(1,0,2)            # (B,T,H)
    cat  = concat([hidden broadcast, enc], -1)         # (B,T,2H)
    en   = tanh(cat @ W_attn.T + b_attn)               # (B,T,H)
    sc   = en @ v                                      # (B,T)
    out  = softmax(sc, axis=1)[:, None, :]             # (B,1,T)

Split W_attn = [W_h | W_e] (each (H,H)):
    q[b]     = hidden[b] @ W_h.T + b_attn              # (B,H) host-precomputed
    E[b,t]   = enc[b,t] @ W_e.T                        # the big matmul
    sc[b,t]  = sum_o v[o] * tanh(q[b,o] + E[b,t,o])

Sharding: data-parallel over B across 8 NeuronCores (4 batches/core),
no collectives. Per-core pipeline (o-chunks on PSUM partitions so q can
ride the ACT bias port):
    mains   PE: E-psum (128 o, 1024 rows) via fp8e4 DoubleRow matmuls
            (2 fp8 weights/cell -> K=256 per matmul, 2 matmuls per
            512-col block; operands pre-scaled on host, descale rides
            the ACT scale immediate)
    tanh    ACT: tanh(E/16384 + q[b, o-chunk]) via per-partition bias
            (the serial ACT pass over all 4.2M elements is the kernel's
            critical path; everything else hides under it)
    scores  PE: per (s,h2) four col-tiled v-window matmuls run
            concurrently in the four 32-column array strips
            (tile_position=(0,32*o)), each contracting one o-chunk's
            tanh tile into rows 32*o + (2s+h2) of a zero-initialized
            (128,512) psum bank; a final DVE copy + one select-matmul
            folds the four strips into the (16,512) softmax layout
    softmax ACT exp with fused accum sum on (16, 512), per-batch
            sum/broadcast via tiny PE matmuls, no max-subtraction
            (scores are bounded by ||v||_1)
Head DMAs: consts+weights ride SWDGE (gpsimd reaches user code ~1.3us
before the HWDGE rings), enc superblock 0 is split across both HWDGE
rings, the rest of enc streams via SWDGE so the ACT queue stays clean
for tanh. Warmup matmuls on a zeroed tile hold the PE clock gate (HAM)
at 2.4 GHz through the DMA head.
"""

import numpy as np
from contextlib import ExitStack

import concourse.bass as bass
import concourse.tile as tile
from concourse import bacc, mybir
import concourse.bass_utils as bass_utils

T, B, H = 2048, 32, 512
NCORES = 8
NB = B // NCORES        # 4 local batches per core
ROWS = NB * T           # 8192 rows per core
P = 128
KP = 2                  # DoubleRow contraction chunks (256 each)
OC = H // P             # 4 output chunks
SUP = 1024              # columns per E-psum tile (2 PSUM banks)
NSUP = ROWS // SUP      # 8
BLK = 512               # matmul moving-dim limit (one PSUM bank)
SE = 32.0               # fp8 scale on enc  (|enc|*32  < 240)
SW = 512.0              # fp8 scale on W_e  (|W|*512 <= 16)
CB = 512 + OC * KP * 2 * P   # head param bytes/partition: consts + wet8
F32 = mybir.dt.float32
F16 = mybir.dt.float16
F8 = mybir.dt.float8e4
AF = mybir.ActivationFunctionType
DR = mybir.MatmulPerfMode.DoubleRow
F8NP = mybir.dt.np(F8)


def _build():
    nc = bacc.Bacc(
        "TRN2", target_bir_lowering=False, debug=False, num_devices=NCORES
    )
    # head[p, 0:512]    = consts row p viewed as fp8 bytes (bitcast to f32):
    #   f32 cols  0:16  qrep[p, o*NB+b] = q[b, o*128+p] (host-computed)
    #   f32 cols 20:24  selb2 (rows 0:16)
    #   f32 cols 24:40  selb  (rows 0:4)
    #   f32 cols 40:48  sel16 as f16[16]: sel[32o+c, c] = 1 (strip fold)
    #   f32 cols 64:128 vwin: 4 chunks of 32 f16 cols, chunk o has
    #                   v[o*128+p] at f16-col 128+32*o+15 (sliding lhsT)
    # head[p, 512:2560] = wet8[p, o, kp, i, m]
    #                   = W_e^T[kp*256 + i*128 + p, o*128 + m] * SW (fp8)
    head = nc.declare_dram_parameter("head", [P, CB], F8, isOutput=False)
    # enc8[p, s, kp, i, t] = enc^T[kp*256 + i*128 + p, s*1024 + t] * SE (fp8)
    enc8 = nc.declare_dram_parameter("enc8", [P, NSUP * KP * 2 * SUP], F8,
                                     isOutput=False)
    out = nc.declare_dram_parameter("out", [NB, T], F32, isOutput=True)

    with tile.TileContext(nc) as tc, ExitStack() as ctx:
        const_pool = ctx.enter_context(tc.tile_pool(name="const", bufs=1))
        enc_pool = ctx.enter_context(tc.tile_pool(name="enc", bufs=1))
        tanh_pool = ctx.enter_context(tc.tile_pool(name="tanh", bufs=8))
        sm_pool = ctx.enter_context(tc.tile_pool(name="sm", bufs=1))
        psE_pool = ctx.enter_context(tc.tile_pool(name="psE", bufs=3, space="PSUM"))
        psS_pool = ctx.enter_context(tc.tile_pool(name="psS", bufs=1, space="PSUM"))

        # constants + weights in one SWDGE transfer (gpsimd queue clears
        # its preamble ~1.3us before the HWDGE rings do)
        h_sb = const_pool.tile([P, CB], F8, tag="head")
        nc.gpsimd.dma_start(h_sb[:], head[:, :])
        c_sb = h_sb[:, 0:512].bitcast(F32)   # (P, 128) f32 view
        c16 = h_sb[:, 0:512].bitcast(F16)    # (P, 256) f16 view
        q_sb = c_sb[:, 0:16]
        selb2_sb = c_sb[0:16, 20:24]
        selb_sb = c_sb[0:4, 24:40]
        sel16_sb = c16[:, 80:96]
        vwin_sb = [c16[:, 128 + 32 * o : 159 + 32 * o] for o in range(OC)]
        # lhsT AP per (o, kp): [128, 2, 128], pair-stride 128 elements
        w8 = h_sb[:, 512:CB].rearrange("p (o k i m) -> p o k i m", o=OC, k=KP, i=2)

        # enc superblock 0 split across the two HWDGE rings so both kp
        # halves land concurrently; later superblocks stream via SWDGE
        # (DVE is idle, so no descriptor-ring port contention) keeping
        # the ACT queue free for the tanh stream.
        SUPB = KP * 2 * SUP  # fp8 bytes per partition per superblock
        enc_sb = [None] * NSUP
        e0 = enc_pool.tile([P, SUPB], F8, tag="e0", name="e0")
        nc.sync.dma_start(e0[:, 0 : 2 * SUP], enc8[:, 0 : 2 * SUP])
        nc.scalar.dma_start(e0[:, 2 * SUP : SUPB], enc8[:, 2 * SUP : SUPB])
        enc_sb[0] = e0

        # PE warmup: matmuls on a zeroed scratch tile run while the first
        # DMAs are still in flight, so the HAM clock gate is already
        # released (2.4 GHz) when real matmuls start.
        warm = const_pool.tile([P, BLK], F16, tag="warm")
        nc.gpsimd.memset(warm[:], 0.0)
        psW = psS_pool.tile([P, BLK], F32, tag="t", name="psW")
        for _ in range(8):
            nc.tensor.matmul(
                psW[:], lhsT=warm[:, 0:P], rhs=warm[:], start=True, stop=True
            )

        for s in range(1, NSUP):
            e = enc_pool.tile([P, SUPB], F8, tag=f"e{s}", name=f"e{s}")
            nc.gpsimd.dma_start(e[:], enc8[:, s * SUPB : (s + 1) * SUPB])
            enc_sb[s] = e

        # score accumulator: rows 32*o + (2s+h2) collect o-chunk partial
        # scores of (batch s//2, t-slice (s%2)*1024 + h2*512); the four
        # o-strips are summed by the sel16 matmul after the main loop.
        # DVE-zeroed once so the never-written rows contract to 0.
        psS = psS_pool.tile([P, BLK], F32, tag="s", name="psS")
        nc.vector.memset(psS[:], 0.0)

        def emit_vdots(ths, s):
            # four col-tiled matmuls per h2 run concurrently in the four
            # 32-column PE strips, one per o-chunk
            for h2 in range(SUP // BLK):
                c = 2 * s + h2
                for o in range(OC):
                    nc.tensor.matmul(
                        psS[32 * o : 32 * o + 16, :],
                        lhsT=vwin_sb[o][:, 15 - c : 31 - c],
                        rhs=ths[o][:, h2 * BLK : (h2 + 1) * BLK],
                        start=False,
                        stop=False,
                        tile_position=(0, 32 * o),
                        skip_group_check=True,
                    )

        pending = None
        for s in range(NSUP):
            b = s // 2
            last_sup = s == NSUP - 1
            es = enc_sb[s][:].rearrange(
                "p (k i t) -> p k i t", k=KP, i=2
            )  # rhs AP per (kp, h2): [128, 2, 512], pair-stride 1024
            ths = []
            for o in range(OC):
                psE = psE_pool.tile([P, SUP], F32, tag="E")
                for h2 in range(SUP // BLK):
                    for kp in range(KP):
                        nc.tensor.matmul(
                            psE[:, h2 * BLK : (h2 + 1) * BLK],
                            lhsT=w8[:, o, kp],
                            rhs=es[:, kp, :, h2 * BLK : (h2 + 1) * BLK],
                            start=(kp == 0),
                            stop=(kp == KP - 1),
                            perf_mode=DR,
                        )
                th = tanh_pool.tile([P, SUP], F16, tag="tanh")
                nc.scalar.activation(
                    th[:],
                    psE[:],
                    AF.Tanh,
                    bias=q_sb[:, o * NB + b : o * NB + b + 1],
                    scale=1.0 / (SE * SW),
                )
                ths.append(th)
                if pending is not None and o == 1:
                    emit_vdots(*pending)
                    pending = None
                if last_sup and o > 0:
                    # eager per-o v-dots so only o=3's trail the last tanh
                    oo = o - 1
                    for h2 in range(SUP // BLK):
                        c = 2 * s + h2
                        nc.tensor.matmul(
                            psS[32 * oo : 32 * oo + 16, :],
                            lhsT=vwin_sb[oo][:, 15 - c : 31 - c],
                            rhs=ths[oo][:, h2 * BLK : (h2 + 1) * BLK],
                            start=False,
                            stop=False,
                            tile_position=(0, 32 * oo),
                            skip_group_check=True,
                        )
            if not last_sup:
                pending = (ths, s)
        for h2 in range(SUP // BLK):
            c = 2 * (NSUP - 1) + h2
            nc.tensor.matmul(
                psS[96:112, :],
                lhsT=vwin_sb[3][:, 15 - c : 31 - c],
                rhs=ths[3][:, h2 * BLK : (h2 + 1) * BLK],
                start=False,
                stop=False,
                tile_position=(0, 96),
                skip_group_check=True,
            )

        # fold the four o-strips: copy psum->sbuf f16, contract with the
        # sel16 lhsT (sel[32o+c, c] = 1) into the (16,512) score tile
        sc_sb = sm_pool.tile([P, BLK], F16, tag="sc")
        nc.vector.tensor_copy(sc_sb[:], psS[:])
        # reuse psS's bank: the copy has drained it, so the folded scores
        # can land in its first 16 partitions
        psS16 = psS[0:16, :]
        nc.tensor.matmul(
            psS16, lhsT=sel16_sb[:], rhs=sc_sb[:], start=True, stop=True,
            skip_group_check=True,
        )

        # softmax on the (16, 512) layout; scores are bounded (|s| <=
        # ||v||_1) so no max-subtraction is needed in f32
        ex16 = sm_pool.tile([16, BLK], F32, tag="ex16")
        sums16 = sm_pool.tile([16, 1], F32, tag="sums16")
        nc.scalar.activation(ex16[:], psS16, AF.Exp, accum_out=sums16[:])
        # per-batch sums: contract the 4 j-rows of each batch on PE
        psT = psS_pool.tile([NB, 1], F32, tag="t", name="psT")
        nc.tensor.matmul(
            psT[:], lhsT=selb2_sb[:], rhs=sums16[:], start=True, stop=True
        )
        rec4 = sm_pool.tile([NB, 1], F32, tag="rec4")
        nc.vector.reciprocal(rec4[:], psT[:])
        # broadcast 1/sum back to the 16 rows
        psB = psS_pool.tile([16, 1], F32, tag="t", name="psB")
        nc.tensor.matmul(
            psB[:], lhsT=selb_sb[:], rhs=rec4[:], start=True, stop=True
        )
        probs16 = sm_pool.tile([16, BLK], F32, tag="probs16")
        nc.vector.tensor_scalar_mul(probs16[:], ex16[:], psB[:, 0:1])
        nc.sync.dma_start(
            out[:, :].rearrange("b (j t) -> (b j) t", j=4), probs16[:]
        )

    nc.compile()
    return nc


_NC = None


def _get_nc():
    global _NC
    if _NC is None:
        _NC = _build()
    return _NC


def _to_f8(x):
    return np.asarray(np.clip(x, -240.0, 240.0), dtype=F8NP)


def _shard_inputs(hidden, encoder_outputs, W_attn, b_attn, v):
    hidden = np.asarray(hidden, dtype=np.float32)
    encoder_outputs = np.asarray(encoder_outputs, dtype=np.float32)
    W_attn = np.asarray(W_attn, dtype=np.float32)
    b_attn = np.asarray(b_attn, dtype=np.float32)
    v = np.asarray(v, dtype=np.float32)

    # wet8[p, o, kp, i, m] = W_e^T[kp*256 + i*128 + p, o*128 + m] * SW
    wet = (W_attn[:, H:].T * SW).reshape(KP, 2, P, OC, P)  # [kp,i,p,o,m]
    wet8 = _to_f8(
        np.ascontiguousarray(wet.transpose(2, 3, 0, 1, 4)).reshape(P, -1)
    )

    # q[b, o] = hidden[b] @ W_h.T + b_attn, computed on host (tiny)
    q = hidden[0] @ W_attn[:, :H].T + b_attn  # (B, H)

    # packed constant block, f32 view (P, 128) / f16 view (P, 256)
    consts = np.zeros((P, 128), dtype=np.float32)
    c16 = consts.view(np.float16)  # (P, 256)
    for b in range(NB):
        for j in range(NB):
            consts[NB * b + j, 20 + b] = 1.0  # selb2 (rows 0:16)
            consts[b, 24 + NB * b + j] = 1.0  # selb (rows 0:4)
    for o in range(OC):
        for c in range(16):
            c16[32 * o + c, 80 + c] = np.float16(1.0)  # sel16 strip fold
    vrT = v.reshape(OC, P).T.astype(np.float16)  # (P, OC)
    for o in range(OC):
        c16[:, 128 + 32 * o + 15] = vrT[:, o]  # vwin sliding windows

    # (H, B, T) so per-core slices are cheap views before the copy
    enc_hbt = np.transpose(encoder_outputs, (2, 1, 0))
    in_maps = []
    for c in range(NCORES):
        b0 = c * NB
        # enc8[p, s, kp, i, t] = enc^T[kp*256+i*128+p, s*1024+t] * SE
        encT = np.ascontiguousarray(
            enc_hbt[:, b0 : b0 + NB, :], dtype=np.float32
        ).reshape(KP, 2, P, NSUP, SUP)  # [kp, i, p, s, t]
        enc8 = _to_f8(
            np.ascontiguousarray(encT.transpose(2, 3, 0, 1, 4) * SE).reshape(
                P, -1
            )
        )
        cc = consts.copy()
        # qrep[p, o*NB+b] = q[b0+b, o*128+p]
        qc = q[b0 : b0 + NB].T.reshape(OC, P, NB)  # [o, p, b]
        cc[:, 0:16] = qc.transpose(1, 0, 2).reshape(P, OC * NB)
        headarr = np.concatenate(
            [cc.view(F8NP), wet8], axis=1
        )  # (P, CB) fp8 bytes
        in_maps.append({"head": headarr, "enc8": enc8})
    return in_maps


def kernel(hidden, encoder_outputs, W_attn, b_attn, v):
    nc = _get_nc()
    in_maps = _shard_inputs(hidden, encoder_outputs, W_attn, b_attn, v)
    res = bass_utils.run_bass_kernel_spmd(
        nc, in_maps, core_ids=list(range(NCORES))
    )
    outs = [res.results[c]["out"] for c in range(NCORES)]  # each (NB, T)
    full = np.concatenate(outs, axis=0)  # (B, T)
    return full[:, None, :].astype(np.float32)  # (B, 1, T)
